# revision 7
# baseline (speedup 1.0000x reference)
"""Trainium2 Bass kernel for nn_Model_22677427323544.

The circuit is AngleEmbedding(adds) followed by a batch-independent gate
sequence, then <Z_0>. Algebraically out[b] = r_b^T A r_b with A a fixed real
symmetric 512x512 matrix and r_b the real Kronecker vector of per-wire
(cos(t/2), sin(t/2)).

Each wire contributes a factor c^2, s^2, or c*s to every A[j,k] r_j r_k term,
so the quadratic form collapses to a LINEAR form over per-wire 3-vectors.
Since (c^2, s^2, c*s) = T (1, cos t, sin t) with a fixed 3x3 T, fold T into
the coefficient tensor on the host: with h_i[b] = (1, cos t_i, sin t_i),

    out[b] = < A3h , h_0[b] x h_1[b] x ... x h_8[b] >

Split wires 0-3 (81) / 4-8 (243):  out[b] = H_hi[b]^T A3h H_lo[b].
The h-basis needs NO device-side work beyond two Sin activations (the ones
plane is a memset), unlike the g-basis which needs squares/products.

Device (per core, 1024 samples = 128 partitions x 8 groups), fp16 pipeline
(validated 5e-4 rel err on host):
  1. cos t = Sin(t + pi/2), sin t = Sin(t) into hv[P,G,3,9] fp16
     (wire slots host-permuted to [0,5,2,7,1,6,3,8,4] so every kron level
     uses contiguous slices)
  2. one DVE op for all four wire-pair krons q[P,G,4,9]; one DVE op for
     ghi[P,G,81] + lo4[P,G,81]; one for glo[P,G,243] = h4 x lo4
  3. per group: PE-transpose ghi (fp16, 1 cyc/row) -> ScalarE PSUM->SBUF
     fp16 cast -> TensorE fp16 matmul Y_g = ghi_g^T @ A3h into one PSUM tile
  4. per group: fused VectorE scalar_tensor_tensor dot rowsum(Y_g * glo_g)
DMAs: adds dispatched on SP, A3h on DVE (parallel dispatch, ~1.5us saved
over serial SP dispatch); out on SP.
"""
import math

import numpy as np

import concourse.bass as bass
import concourse.tile as tile
from concourse import bacc, mybir
from concourse import bass_utils

N_WIRES = 9
N_CORES = 8
B = 8192
B_LOC = B // N_CORES          # 1024
P = 128                       # partitions
G = B_LOC // P                # 8 batch groups per partition
NH = 81                       # 3^4, wires 0-3
NL = 243                      # 3^5, wires 4-8
NLP = 256                     # padded so each PSUM matmul slab is half a bank
F32 = mybir.dt.float32
F16 = mybir.dt.float16

# wire -> hv slot order: slots 0-3 = hi factors of pairs (0,1),(5,6),(2,3),
# (7,8); slots 4-7 = lo factors; slot 8 = wire 4.  This makes every kron
# level read contiguous slot slices (see _build_program).
PERM = [0, 5, 2, 7, 1, 6, 3, 8, 4]

# ---------------------------------------------------------------------------
# Host-side parameter folding: A = Re(D^H U^H Z0 U D), 3-ary fold, T fold
# ---------------------------------------------------------------------------

DIM = 1 << N_WIRES

_X = np.array([[0, 1], [1, 0]], dtype=np.complex128)
_CNOT = np.array(
    [[1, 0, 0, 0], [0, 1, 0, 0], [0, 0, 0, 1], [0, 0, 1, 0]], dtype=np.complex128
)


def _rx(t):
    c, s = np.cos(t / 2), np.sin(t / 2)
    return np.array([[c, -1j * s], [-1j * s, c]])


def _ry(t):
    c, s = np.cos(t / 2), np.sin(t / 2)
    return np.array([[c, -s], [s, c]], dtype=np.complex128)


def _rz(t):
    return np.array([[np.exp(-0.5j * t), 0], [0, np.exp(0.5j * t)]])


def _rot(phi, theta, omega):
    return _rz(omega) @ _ry(theta) @ _rz(phi)


def _crz(t):
    return np.diag([1, 1, np.exp(-0.5j * t), np.exp(0.5j * t)]).astype(np.complex128)


def _crx(t):
    m = np.eye(4, dtype=np.complex128)
    m[2:, 2:] = _rx(t)
    return m


def _apply_1q(state, U, w):
    s = np.moveaxis(state, 1 + w, -1)
    s = np.einsum('ij,...j->...i', U, s)
    return np.moveaxis(s, -1, 1 + w)


def _apply_2q(state, U, c, t):
    s = np.moveaxis(state, (1 + c, 1 + t), (-2, -1))
    shp = s.shape
    s = s.reshape(shp[:-2] + (4,))
    s = np.einsum('ij,...j->...i', U, s)
    return np.moveaxis(s.reshape(shp), (-2, -1), (1 + c, 1 + t))


def _entangle_block(state, p):
    j = 0
    for i in range(N_WIRES):
        ip = (i + 1) % N_WIRES
        state = _apply_1q(state, _ry(p[j]), i)
        state = _apply_1q(state, _ry(p[j + 1]), ip)
        state = _apply_2q(state, _CNOT, i, ip)
        state = _apply_2q(state, _crz(p[j + 2]), i, ip)
        state = _apply_1q(state, _X, ip)
        state = _apply_2q(state, _crx(p[j + 3]), i, ip)
        j += 4
    return state


def _sel_layer(state, w, r):
    for i in range(N_WIRES):
        state = _apply_1q(state, _rot(w[i, 0], w[i, 1], w[i, 2]), i)
    for i in range(N_WIRES):
        state = _apply_2q(state, _CNOT, i, (i + r) % N_WIRES)
    return state


def _compute_A(params, weights, params2):
    """Return the folded h-basis coefficient matrix A3h [81, 256] (fp16)."""
    params = np.asarray(params, np.float64)
    weights = np.asarray(weights, np.float64)
    params2 = np.asarray(params2, np.float64)
    state = np.eye(DIM, dtype=np.complex128).reshape((DIM,) + (2,) * N_WIRES)
    for l in range(3):
        state = _entangle_block(state, params[l * 36:(l + 1) * 36])
    for l in range(3):
        state = _sel_layer(state, weights[l], (l % (N_WIRES - 1)) + 1)
    for l in range(5):
        state = _entangle_block(state, params2[l * 36:(l + 1) * 36])
    U = state.reshape(DIM, DIM).T
    z = np.where(np.arange(DIM) < DIM // 2, 1.0, -1.0)
    M = U.conj().T @ (z[:, None] * U)
    pc = np.array([bin(j).count('1') for j in range(DIM)])
    d = (-1j) ** pc
    A = ((np.conj(d)[:, None] * M * d[None, :]).real).astype(np.float64)

    # fold 512x512 -> 3^9: digit 0 = (0,0), 1 = (1,1), 2 = (0,1)/(1,0)
    j = np.arange(DIM)
    jb = (j[:, None, None] >> (8 - np.arange(N_WIRES))[None, None, :]) & 1
    kb = (j[None, :, None] >> (8 - np.arange(N_WIRES))[None, None, :]) & 1
    digit = np.where((jb == 0) & (kb == 0), 0, np.where((jb == 1) & (kb == 1), 1, 2))
    m = np.zeros((DIM, DIM), np.int64)
    for i in range(N_WIRES):
        m = m * 3 + digit[:, :, i]
    A3 = np.zeros(3 ** N_WIRES)
    np.add.at(A3, m.ravel(), A.ravel())

    # change of basis per wire: g = (c^2, s^2, cs) = T (1, cos t, sin t)
    T = np.array([[.5, .5, 0.], [.5, -.5, 0.], [0., 0., .5]])
    A9 = A3.reshape((3,) * N_WIRES)
    for ax in range(N_WIRES):
        A9 = np.moveaxis(np.tensordot(A9, T, axes=([ax], [0])), -1, ax)
    A3h = A9.reshape(NH, NL)
    A3p = np.zeros((NH, NLP), np.float16)
    A3p[:, :NL] = A3h.astype(np.float16)
    return np.ascontiguousarray(A3p)


# ---------------------------------------------------------------------------
# Device program (per core: 1024 samples; sample index = p*G + g)
# ---------------------------------------------------------------------------

_PROGRAM = None


def _build_program():
    nc = bacc.Bacc("TRN2", target_bir_lowering=False, debug=False,
                   num_devices=N_CORES)
    adds_ext = nc.dram_tensor("adds", [B_LOC, N_WIRES], F32,
                              kind="ExternalInput").ap()
    amat_ext = nc.dram_tensor("amat", [NH, NLP], F16,
                              kind="ExternalInput").ap()
    out_ext = nc.dram_tensor("out", [B_LOC], F32, kind="ExternalOutput").ap()

    SIN = mybir.ActivationFunctionType.Sin
    N_WARM = 14  # PE warm-up matmuls (~4.5us continuous -> full PE clock)

    with tile.TileContext(nc) as tc:
        with (
            tc.tile_pool(name="const", bufs=1) as cpool,
            tc.tile_pool(name="psum_t", bufs=2, space="PSUM") as pt,
            tc.tile_pool(name="psum_w", bufs=1, space="PSUM") as pw,
            tc.tile_pool(name="psum_y", bufs=1, space="PSUM") as py,
        ):
            # input DMAs: adds on SP, A3h on Scalar -- parallel dispatch
            adds_sb = cpool.tile([P, G, N_WIRES], F32)
            nc.sync.dma_start(adds_sb[:], adds_ext.rearrange("(p g) i -> p g i", g=G))
            a3_sb = cpool.tile([NH, NLP], F16)
            nc.scalar.dma_start(a3_sb[:], amat_ext)

            # identity for PE transpose (fp32 to match transposed data)
            ident = cpool.tile([P, P], F32)
            nc.gpsimd.memset(ident[:], 0.0)
            nc.gpsimd.affine_select(
                out=ident[:], in_=ident[:],
                compare_op=mybir.AluOpType.not_equal, fill=1.0,
                base=0, pattern=[[-1, P]], channel_multiplier=1)

            # hv[p,g,comp,slot]: comp 0 = 1, 1 = cos t, 2 = sin t
            hv = cpool.tile([P, G, 3, N_WIRES], F32)
            nc.gpsimd.memset(hv[:, :, 0, :], 1.0)

            # dummy activation on a ready tile: hoists the Sin ACT_TABLE_LOAD
            # off the adds-DMA critical path
            sdum = cpool.tile([P, 1], F32)
            nc.scalar.activation(sdum[:], ident[:, 0:1], SIN, scale=1.0)

            # PE warm-up: ~N_WARM dummy transposes keep the PE busy through
            # the DMA wait so the clock is ramped when real work arrives
            warm = pw.tile([P, P], F32)
            for _ in range(N_WARM):
                nc.tensor.transpose(warm[:], ident[:], ident[:])

            # half-angle trig (Sin inputs stay within the proven table range):
            # w = sin(t/2), u = sin(t/4); cos t = 1-2w^2,
            # sin t = 2w(1-2u^2)
            w = cpool.tile([P, G, N_WIRES], F32)
            u = cpool.tile([P, G, N_WIRES], F32)
            nc.scalar.activation(w[:], adds_sb[:], SIN, scale=0.5)
            nc.scalar.activation(u[:], adds_sb[:], SIN, scale=0.25)
            wsq = cpool.tile([P, G, N_WIRES], F32)
            usq = cpool.tile([P, G, N_WIRES], F32)
            c2 = cpool.tile([P, G, N_WIRES], F32)
            nc.vector.tensor_mul(wsq[:], w[:], w[:])
            nc.vector.tensor_mul(usq[:], u[:], u[:])
            nc.vector.tensor_scalar(
                out=hv[:, :, 1, :], in0=wsq[:], scalar1=-2.0, scalar2=1.0,
                op0=mybir.AluOpType.mult, op1=mybir.AluOpType.add)
            nc.vector.tensor_scalar(
                out=c2[:], in0=usq[:], scalar1=-2.0, scalar2=1.0,
                op0=mybir.AluOpType.mult, op1=mybir.AluOpType.add)
            nc.vector.scalar_tensor_tensor(
                out=hv[:, :, 2, :], in0=w[:], scalar=2.0, in1=c2[:],
                op0=mybir.AluOpType.mult, op1=mybir.AluOpType.mult)

            # q[p,g,j,3b+m] = hv[p,g,b,j] * hv[p,g,m,4+j], one DVE op per b
            # (DVE ISA caps free dims at 3).  j order: pairs (w0,w1),(w5,w6),
            # (w2,w3),(w7,w8).
            q = cpool.tile([P, G, 4, 9], F32)
            q_lo = hv[:, :, :, 4:8].rearrange("p g m j -> p g j m")
            for b in range(3):
                q_out = q[:].rearrange("p g j (b m) -> p g j b m", b=3)[:, :, :, b, :]
                q_hi = hv[:, :, b, 0:4][:, :, :, None].to_broadcast((P, G, 4, 3))
                nc.vector.tensor_mul(q_out, q_hi, q_lo)

            # rr[p,g,0,9B+M] = q0[B]*q2[M] = ghi (digits d0d1 d2d3)
            # rr[p,g,1,9B+M] = q1[B]*q3[M] = lo4 (digits d5d6 d7d8)
            rr = cpool.tile([P, G, 2, NH], F32)
            for k in range(2):
                rr_out = rr[:, :, k, :].rearrange("p g (B M) -> p g B M", B=9)
                rr_hi = q[:, :, k, :][:, :, :, None].to_broadcast((P, G, 9, 9))
                rr_lo = q[:, :, 2 + k, :][:, :, None, :].to_broadcast((P, G, 9, 9))
                nc.vector.tensor_mul(rr_out, rr_hi, rr_lo)

            # per 2 groups: PE transpose ghi_g, cast fp32->fp16, matmul early
            ghiT = cpool.tile([NH, G, P], F16)
            yp = py.tile([P, G, NLP], F32)
            for pair in range(4):
                tp = pt.tile([NH, 2, P], F32, tag="tp")
                for qq in range(2):
                    g = pair * 2 + qq
                    nc.tensor.transpose(tp[:, qq, :], rr[:, g, 0, :], ident[:])
                nc.scalar.copy(ghiT[:, pair * 2:pair * 2 + 2, :], tp[:])

            # glo[p,g,81c+M] = hv[p,g,c,8] * lo4[p,g,M]
            # groups 0-3 on DVE, 4-7 on GpSimd (parallel engines)
            glo = cpool.tile([P, G, NL], F32)
            glo_out = glo[:].rearrange("p g (c M) -> p g c M", c=3)
            glo_hi = hv[:, :, :, 8][:, :, :, None].to_broadcast((P, G, 3, NH))
            glo_lo = rr[:, :, 1, :][:, :, None, :].to_broadcast((P, G, 3, NH))
            nc.vector.tensor_mul(glo_out[:, 0:4], glo_hi[:, 0:4], glo_lo[:, 0:4])
            nc.gpsimd.tensor_mul(glo_out[:, 4:8], glo_hi[:, 4:8], glo_lo[:, 4:8])

            for g in range(G):
                nc.tensor.matmul(yp[:, g, :], lhsT=ghiT[:, g, :], rhs=a3_sb[:],
                                 start=True, stop=True)

            # out[:, g] = rowsum(Y_g * glo_g), fused
            res = cpool.tile([P, G], F32)
            wscr0 = cpool.tile([P, NL], F32)
            wscr1 = cpool.tile([P, NL], F32)
            for g in range(G):
                wscr = wscr0 if g % 2 == 0 else wscr1
                nc.vector.scalar_tensor_tensor(
                    out=wscr[:], in0=glo[:, g, :], scalar=0.0,
                    in1=yp[:, g, 0:NL],
                    op0=mybir.AluOpType.add, op1=mybir.AluOpType.mult,
                    accum_out=res[:, g:g + 1])

            nc.sync.dma_start(out_ext.rearrange("(p g) -> p g", g=G), res[:])

    nc.compile()
    return nc


def _get_program():
    global _PROGRAM
    if _PROGRAM is None:
        _PROGRAM = _build_program()
    return _PROGRAM


def kernel(adds, params, weights, params2):
    adds = np.ascontiguousarray(np.asarray(adds)[:, PERM], dtype=np.float32)
    A = _compute_A(params, weights, params2)
    nc = _get_program()
    in_maps = [
        {"adds": adds[i * B_LOC:(i + 1) * B_LOC], "amat": A}
        for i in range(N_CORES)
    ]
    results = bass_utils.run_bass_kernel_spmd(nc, in_maps, list(range(N_CORES))).results
    return np.concatenate([results[i]["out"] for i in range(N_CORES)])


# revision 11
# speedup vs baseline: 1.1235x; 1.1235x over previous
"""Trainium2 Bass kernel for nn_Model_22677427323544.

The circuit is AngleEmbedding(adds) followed by a batch-independent gate
sequence, then <Z_0>. Algebraically out[b] = r_b^T A r_b with A a fixed real
symmetric 512x512 matrix and r_b the real Kronecker vector of per-wire
(cos(t/2), sin(t/2)).

Each wire contributes a factor c^2, s^2, or c*s to every A[j,k] r_j r_k term,
so the quadratic form collapses to a LINEAR form over per-wire 3-vectors.
Since (c^2, s^2, c*s) = T (1, cos t, sin t) with a fixed 3x3 T, fold T into
the coefficient tensor on the host: with h_i[b] = (1, cos t_i, sin t_i),

    out[b] = < A3h , h_0[b] x h_1[b] x ... x h_8[b] >

Split wires 0-3 (81) / 4-8 (243):  out[b] = H_hi[b]^T A3h H_lo[b].
The h-basis needs NO device-side work beyond two Sin activations (the ones
plane is a memset), unlike the g-basis which needs squares/products.

Device (per core, 1024 samples = 128 partitions x 8 groups), fp16 pipeline
(validated 5e-4 rel err on host):
  1. cos t = Sin(t + pi/2), sin t = Sin(t) into hv[P,G,3,9] fp16
     (wire slots host-permuted to [0,5,2,7,1,6,3,8,4] so every kron level
     uses contiguous slices)
  2. one DVE op for all four wire-pair krons q[P,G,4,9]; one DVE op for
     ghi[P,G,81] + lo4[P,G,81]; one for glo[P,G,243] = h4 x lo4
  3. per group: PE-transpose ghi (fp16, 1 cyc/row) -> ScalarE PSUM->SBUF
     fp16 cast -> TensorE fp16 matmul Y_g = ghi_g^T @ A3h into one PSUM tile
  4. per group: fused VectorE scalar_tensor_tensor dot rowsum(Y_g * glo_g)
DMAs: adds dispatched on SP, A3h on DVE (parallel dispatch, ~1.5us saved
over serial SP dispatch); out on SP.
"""
import math

import numpy as np

import concourse.bass as bass
import concourse.tile as tile
from concourse import bacc, mybir
from concourse import bass_utils

N_WIRES = 9
N_CORES = 8
B = 8192
B_LOC = B // N_CORES          # 1024
P = 128                       # partitions
G = B_LOC // P                # 8 batch groups per partition
NH = 81                       # 3^4, wires 0-3
NL = 243                      # 3^5, wires 4-8
NLP = 256                     # padded so each PSUM matmul slab is half a bank
F32 = mybir.dt.float32
F16 = mybir.dt.float16

# wire -> hv slot order: slots 0-3 = hi factors of pairs (0,1),(5,6),(2,3),
# (7,8); slots 4-7 = lo factors; slot 8 = wire 4.  This makes every kron
# level read contiguous slot slices (see _build_program).
PERM = [0, 5, 2, 7, 1, 6, 3, 8, 4]

# ---------------------------------------------------------------------------
# Host-side parameter folding: A = Re(D^H U^H Z0 U D), 3-ary fold, T fold
# ---------------------------------------------------------------------------

DIM = 1 << N_WIRES

_X = np.array([[0, 1], [1, 0]], dtype=np.complex128)
_CNOT = np.array(
    [[1, 0, 0, 0], [0, 1, 0, 0], [0, 0, 0, 1], [0, 0, 1, 0]], dtype=np.complex128
)


def _rx(t):
    c, s = np.cos(t / 2), np.sin(t / 2)
    return np.array([[c, -1j * s], [-1j * s, c]])


def _ry(t):
    c, s = np.cos(t / 2), np.sin(t / 2)
    return np.array([[c, -s], [s, c]], dtype=np.complex128)


def _rz(t):
    return np.array([[np.exp(-0.5j * t), 0], [0, np.exp(0.5j * t)]])


def _rot(phi, theta, omega):
    return _rz(omega) @ _ry(theta) @ _rz(phi)


def _crz(t):
    return np.diag([1, 1, np.exp(-0.5j * t), np.exp(0.5j * t)]).astype(np.complex128)


def _crx(t):
    m = np.eye(4, dtype=np.complex128)
    m[2:, 2:] = _rx(t)
    return m


def _apply_1q(state, U, w):
    s = np.moveaxis(state, 1 + w, -1)
    s = np.einsum('ij,...j->...i', U, s)
    return np.moveaxis(s, -1, 1 + w)


def _apply_2q(state, U, c, t):
    s = np.moveaxis(state, (1 + c, 1 + t), (-2, -1))
    shp = s.shape
    s = s.reshape(shp[:-2] + (4,))
    s = np.einsum('ij,...j->...i', U, s)
    return np.moveaxis(s.reshape(shp), (-2, -1), (1 + c, 1 + t))


def _entangle_block(state, p):
    j = 0
    for i in range(N_WIRES):
        ip = (i + 1) % N_WIRES
        state = _apply_1q(state, _ry(p[j]), i)
        state = _apply_1q(state, _ry(p[j + 1]), ip)
        state = _apply_2q(state, _CNOT, i, ip)
        state = _apply_2q(state, _crz(p[j + 2]), i, ip)
        state = _apply_1q(state, _X, ip)
        state = _apply_2q(state, _crx(p[j + 3]), i, ip)
        j += 4
    return state


def _sel_layer(state, w, r):
    for i in range(N_WIRES):
        state = _apply_1q(state, _rot(w[i, 0], w[i, 1], w[i, 2]), i)
    for i in range(N_WIRES):
        state = _apply_2q(state, _CNOT, i, (i + r) % N_WIRES)
    return state


def _compute_A(params, weights, params2):
    """Return the folded h-basis coefficient matrix A3h [81, 256] (fp16)."""
    params = np.asarray(params, np.float64)
    weights = np.asarray(weights, np.float64)
    params2 = np.asarray(params2, np.float64)
    state = np.eye(DIM, dtype=np.complex128).reshape((DIM,) + (2,) * N_WIRES)
    for l in range(3):
        state = _entangle_block(state, params[l * 36:(l + 1) * 36])
    for l in range(3):
        state = _sel_layer(state, weights[l], (l % (N_WIRES - 1)) + 1)
    for l in range(5):
        state = _entangle_block(state, params2[l * 36:(l + 1) * 36])
    U = state.reshape(DIM, DIM).T
    z = np.where(np.arange(DIM) < DIM // 2, 1.0, -1.0)
    M = U.conj().T @ (z[:, None] * U)
    pc = np.array([bin(j).count('1') for j in range(DIM)])
    d = (-1j) ** pc
    A = ((np.conj(d)[:, None] * M * d[None, :]).real).astype(np.float64)

    # fold 512x512 -> 3^9: digit 0 = (0,0), 1 = (1,1), 2 = (0,1)/(1,0)
    j = np.arange(DIM)
    jb = (j[:, None, None] >> (8 - np.arange(N_WIRES))[None, None, :]) & 1
    kb = (j[None, :, None] >> (8 - np.arange(N_WIRES))[None, None, :]) & 1
    digit = np.where((jb == 0) & (kb == 0), 0, np.where((jb == 1) & (kb == 1), 1, 2))
    m = np.zeros((DIM, DIM), np.int64)
    for i in range(N_WIRES):
        m = m * 3 + digit[:, :, i]
    A3 = np.zeros(3 ** N_WIRES)
    np.add.at(A3, m.ravel(), A.ravel())

    # change of basis per wire: g = (c^2, s^2, cs) = T (1, cos t, sin t)
    T = np.array([[.5, .5, 0.], [.5, -.5, 0.], [0., 0., .5]])
    A9 = A3.reshape((3,) * N_WIRES)
    for ax in range(N_WIRES):
        A9 = np.moveaxis(np.tensordot(A9, T, axes=([ax], [0])), -1, ax)
    A3h = A9.reshape(NH, NL)
    A3p = np.zeros((NH, NLP), np.float16)
    A3p[:, :NL] = A3h.astype(np.float16)
    return np.ascontiguousarray(A3p)


# ---------------------------------------------------------------------------
# Device program (per core: 1024 samples; sample index = p*G + g)
# ---------------------------------------------------------------------------

_PROGRAM = None


def _build_program():
    nc = bacc.Bacc("TRN2", target_bir_lowering=False, debug=False,
                   num_devices=N_CORES)
    adds_ext = nc.dram_tensor("adds", [B_LOC, N_WIRES], F32,
                              kind="ExternalInput").ap()
    amat_ext = nc.dram_tensor("amat", [NH, NLP], F16,
                              kind="ExternalInput").ap()
    out_ext = nc.dram_tensor("out", [B_LOC], F32, kind="ExternalOutput").ap()

    SIN = mybir.ActivationFunctionType.Sin

    with tile.TileContext(nc) as tc:
        with (
            tc.tile_pool(name="const", bufs=1) as cpool,
            tc.tile_pool(name="psum_t", bufs=2, space="PSUM") as pt,
            tc.tile_pool(name="psum_y", bufs=4, space="PSUM") as py,
        ):
            # input DMAs: adds on SP, A3h on Scalar -- parallel dispatch
            adds_sb = cpool.tile([P, G, N_WIRES], F32)
            nc.sync.dma_start(adds_sb[:], adds_ext.rearrange("(p g) i -> p g i", g=G))
            a3_sb = cpool.tile([NH, NLP], F16)
            nc.scalar.dma_start(a3_sb[:], amat_ext)

            # identity for PE transpose (fp32 to match transposed data)
            ident = cpool.tile([P, P], F32)
            nc.gpsimd.memset(ident[:], 0.0)
            nc.gpsimd.affine_select(
                out=ident[:], in_=ident[:],
                compare_op=mybir.AluOpType.not_equal, fill=1.0,
                base=0, pattern=[[-1, P]], channel_multiplier=1)

            # hv[p,g,comp,slot]: comp 0 = 1, 1 = cos t, 2 = sin t
            hv = cpool.tile([P, G, 3, N_WIRES], F32)
            nc.gpsimd.memset(hv[:, :, 0, :], 1.0)

            # dummy activation on a ready tile: hoists the Sin ACT_TABLE_LOAD
            # off the adds-DMA critical path
            sdum = cpool.tile([P, 1], F32)
            nc.scalar.activation(sdum[:], ident[:, 0:1], SIN, scale=1.0)

            # half-angle trig (Sin inputs stay within the proven table range):
            # w = sin(t/2), u = sin(t/4); cos t = 1-2w^2,
            # sin t = 2w(1-2u^2)
            w = cpool.tile([P, G, N_WIRES], F32)
            u = cpool.tile([P, G, N_WIRES], F32)
            nc.scalar.activation(w[:], adds_sb[:], SIN, scale=0.5)
            nc.scalar.activation(u[:], adds_sb[:], SIN, scale=0.25)
            wsq = cpool.tile([P, G, N_WIRES], F32)
            usq = cpool.tile([P, G, N_WIRES], F32)
            c2 = cpool.tile([P, G, N_WIRES], F32)
            nc.vector.tensor_mul(wsq[:], w[:], w[:])
            nc.vector.tensor_mul(usq[:], u[:], u[:])
            nc.vector.tensor_scalar(
                out=hv[:, :, 1, :], in0=wsq[:], scalar1=-2.0, scalar2=1.0,
                op0=mybir.AluOpType.mult, op1=mybir.AluOpType.add)
            nc.vector.tensor_scalar(
                out=c2[:], in0=usq[:], scalar1=-2.0, scalar2=1.0,
                op0=mybir.AluOpType.mult, op1=mybir.AluOpType.add)
            nc.vector.scalar_tensor_tensor(
                out=hv[:, :, 2, :], in0=w[:], scalar=2.0, in1=c2[:],
                op0=mybir.AluOpType.mult, op1=mybir.AluOpType.mult)

            # q[p,g,j,3b+m] = hv[p,g,b,j] * hv[p,g,m,4+j], one DVE op per b
            # (DVE ISA caps free dims at 3).  j order: pairs (w0,w1),(w5,w6),
            # (w2,w3),(w7,w8).
            q = cpool.tile([P, G, 4, 9], F32)
            q_lo = hv[:, :, :, 4:8].rearrange("p g m j -> p g j m")
            for b in range(3):
                q_out = q[:].rearrange("p g j (b m) -> p g j b m", b=3)[:, :, :, b, :]
                q_hi = hv[:, :, b, 0:4][:, :, :, None].to_broadcast((P, G, 4, 3))
                nc.vector.tensor_mul(q_out, q_hi, q_lo)

            # rr[p,g,0,9B+M] = q0[B]*q2[M] = ghi (digits d0d1 d2d3)
            # rr[p,g,1,9B+M] = q1[B]*q3[M] = lo4 (digits d5d6 d7d8)
            rr = cpool.tile([P, G, 2, NH], F32)
            for k in range(2):
                rr_out = rr[:, :, k, :].rearrange("p g (B M) -> p g B M", B=9)
                rr_hi = q[:, :, k, :][:, :, :, None].to_broadcast((P, G, 9, 9))
                rr_lo = q[:, :, 2 + k, :][:, :, None, :].to_broadcast((P, G, 9, 9))
                nc.vector.tensor_mul(rr_out, rr_hi, rr_lo)

            # per 2 groups: PE transpose ghi_g, cast fp32->fp16, matmul early
            ghiT = cpool.tile([NH, G, P], F16)
            for pair in range(4):
                tp = pt.tile([NH, 2, P], F32, tag="tp")
                for qq in range(2):
                    g = pair * 2 + qq
                    nc.tensor.transpose(tp[:, qq, :], rr[:, g, 0, :], ident[:])
                nc.scalar.copy(ghiT[:, pair * 2:pair * 2 + 2, :], tp[:])

            # glo[p,g,81c+M] = hv[p,g,c,8] * lo4[p,g,M]
            # groups 0-3 on DVE, 4-7 on GpSimd (parallel engines)
            glo = cpool.tile([P, G, NL], F32)
            glo_out = glo[:].rearrange("p g (c M) -> p g c M", c=3)
            glo_hi = hv[:, :, :, 8][:, :, :, None].to_broadcast((P, G, 3, NH))
            glo_lo = rr[:, :, 1, :][:, :, None, :].to_broadcast((P, G, 3, NH))
            nc.vector.tensor_mul(glo_out[:, 0:4], glo_hi[:, 0:4], glo_lo[:, 0:4])
            nc.gpsimd.tensor_mul(glo_out[:, 4:8], glo_hi[:, 4:8], glo_lo[:, 4:8])

            # separate PSUM tiles per group so each STT depends only on its
            # own matmul (a single shared tile would serialize on all 8)
            yps = []
            for g in range(G):
                yp = py.tile([P, NLP], F32, tag="yp")
                nc.tensor.matmul(yp[:], lhsT=ghiT[:, g, :], rhs=a3_sb[:],
                                 start=True, stop=True)
                yps.append(yp)

            # out[:, g] = rowsum(Y_g * glo_g), fused; ship each half as soon
            # as it is done (overlaps DMA latency with remaining STTs)
            res = cpool.tile([P, G], F32)
            wscr0 = cpool.tile([P, NL], F32)
            wscr1 = cpool.tile([P, NL], F32)
            out_pg = out_ext.rearrange("(p g) -> p g", g=G)
            for g in range(G):
                wscr = wscr0 if g % 2 == 0 else wscr1
                nc.vector.scalar_tensor_tensor(
                    out=wscr[:], in0=glo[:, g, :], scalar=0.0,
                    in1=yps[g][:, 0:NL],
                    op0=mybir.AluOpType.add, op1=mybir.AluOpType.mult,
                    accum_out=res[:, g:g + 1])
                if g == 3:
                    nc.sync.dma_start(out_pg[:, 0:4], res[:, 0:4])
            nc.sync.dma_start(out_pg[:, 4:8], res[:, 4:8])

    nc.compile()
    return nc


def _get_program():
    global _PROGRAM
    if _PROGRAM is None:
        _PROGRAM = _build_program()
    return _PROGRAM


def kernel(adds, params, weights, params2):
    adds = np.ascontiguousarray(np.asarray(adds)[:, PERM], dtype=np.float32)
    A = _compute_A(params, weights, params2)
    nc = _get_program()
    in_maps = [
        {"adds": adds[i * B_LOC:(i + 1) * B_LOC], "amat": A}
        for i in range(N_CORES)
    ]
    results = bass_utils.run_bass_kernel_spmd(nc, in_maps, list(range(N_CORES))).results
    return np.concatenate([results[i]["out"] for i in range(N_CORES)])
